# revision 13
# baseline (speedup 1.0000x reference)
"""Self-contained Trainium2 Bass kernel for the GPT forward pass.

B=2, T=2048, V=32000, E=1024, H=16, HS=64, L=8 -> (logits [4096,32000], loss).

Strategy: sequence-data-parallel over 8 NeuronCores. Each core owns 512
tokens (zigzag chunks r and 15-r of each sample for causal balance) and
holds a full replica of the weights (pre-cast to bf16 on host). Per layer
one AllGather ships K/V (bf16). The LM head is vocab-sharded (4000
columns per core) after an AllGather of the final-LN activations.
Activations are kept transposed ([E, tok]) in SBUF; LayerNorm statistics
use ones-matmuls over the partition dim; softmax uses exp without max
subtraction (scores are bounded because inputs are LayerNormed); causal
masks arrive as per-core host data so the SPMD graph is identical on all
cores; a ones column appended to V produces the softmax denominator for
free.
"""

import math
import sys

import numpy as np
import ml_dtypes

sys.path.insert(0, "/opt/trn_rl_repo")

import concourse.bass as bass  # noqa: E402
import concourse.mybir as mybir  # noqa: E402
from concourse import bacc, tile  # noqa: E402
from concourse.bass_utils import run_bass_kernel_spmd  # noqa: E402

F32 = mybir.dt.float32
F32R = mybir.dt.float32r
BF16 = mybir.dt.bfloat16
EXPF = mybir.ActivationFunctionType.Exp
IDENT = mybir.ActivationFunctionType.Identity
RELU = mybir.ActivationFunctionType.Relu
SQUARE = mybir.ActivationFunctionType.Square
SQRT = mybir.ActivationFunctionType.Sqrt
ADD = mybir.AluOpType.add

B, T, V, E, H, L = 2, 2048, 32000, 1024, 16, 8
HS = E // H          # 64
NCORE = 8
TOK = 512            # tokens per core: 2 samples x 2 chunks x 128
VS = V // NCORE      # 4000 vocab per core
NCH = T // 128       # 16 chunks per sample
KT = E // 128        # 8 k-tiles over E
EPS = 1e-5
NEG = -1.0e30

# kv_send layout (elements, bf16):
#   k part: [s(2)][hp(8)][128 x 256]            -> 2*8*32768   = 524288
#   v part: [s(2)][slot(2)][hp(8)][128 x 130]   -> 2*2*8*16640 = 532480
KOFF = 0
VOFF = 2 * 8 * 32768
NRANK = VOFF + 2 * 2 * 8 * 16640  # 1056768 elems per rank


def _koff(s, hp):
    return KOFF + s * 8 * 32768 + hp * 32768


def _voff(s, slot, hp):
    return VOFF + s * (2 * 8 * 16640) + slot * (8 * 16640) + hp * 16640


# lnp packed param columns (f32, [L, 128, 80]):
C_LN1W, C_LN1B, C_LN2W, C_LN2B, C_BO, C_B2, C_B1 = 0, 8, 16, 24, 32, 40, 48


def build(n_layers=L, debug_xt=False):
    nc = bacc.Bacc(None, num_devices=NCORE)

    x0t = nc.declare_dram_parameter("x0t", [KT, 128, TOK], F32, isOutput=False)
    wq = nc.declare_dram_parameter("wq", [L, E, E], BF16, isOutput=False)
    wk = nc.declare_dram_parameter("wk", [L, E, E], BF16, isOutput=False)
    wv = nc.declare_dram_parameter("wv", [L, E, E], BF16, isOutput=False)
    wo = nc.declare_dram_parameter("wo", [L, E, E], BF16, isOutput=False)
    w1 = nc.declare_dram_parameter("w1", [L, E, 4 * E], BF16, isOutput=False)
    w2 = nc.declare_dram_parameter("w2", [L, 4 * E, E], BF16, isOutput=False)
    lnp = nc.declare_dram_parameter("lnp", [L, 128, 80], F32, isOutput=False)
    lnfp = nc.declare_dram_parameter("lnfp", [128, 16], F32, isOutput=False)
    lmw = nc.declare_dram_parameter("lmw", [E, VS], BF16, isOutput=False)
    lmb = nc.declare_dram_parameter("lmb", [8, 500], F32, isOutput=False)
    maska = nc.declare_dram_parameter("maska", [128, 1024], F32, isOutput=False)
    maskbt = nc.declare_dram_parameter("maskbt", [128, 1024], F32, isOutput=False)
    tri2 = nc.declare_dram_parameter("tri2", [128, 256], F32, isOutput=False)

    logits_o = nc.declare_dram_parameter("logits_s", [NCORE * TOK, VS], F32,
                                         isOutput=True)
    sexp_o = nc.declare_dram_parameter("sexp", [32, 128], F32, isOutput=True)
    if debug_xt:
        dbg_o = nc.declare_dram_parameter("dbg_xt", [KT, 128, TOK], F32,
                                          isOutput=True)

    kv_send = nc.dram_tensor("kv_send", [NRANK], BF16)
    kv_gath = nc.dram_tensor("kv_gath", [NCORE * NRANK], BF16,
                             addr_space="Shared")
    xf_send = nc.dram_tensor("xf_send", [KT * 128 * TOK], BF16)
    xf_gath = nc.dram_tensor("xf_gath", [NCORE * KT * 128 * TOK], BF16,
                             addr_space="Shared")

    from contextlib import ExitStack
    with tile.TileContext(nc) as tc:
        with ExitStack() as ctx:
            def pool(name, bufs, space="SBUF"):
                return ctx.enter_context(
                    tc.tile_pool(name=name, bufs=bufs, space=space))
            p_x = pool("p_x", 8)
            p_ln = pool("p_ln", 16)
            p_h = pool("p_h", 8)
            p_qk = pool("p_qk", 16)
            p_vloc = pool("p_vloc", 32)
            p_oT = pool("p_oT", 8)
            p_mT = pool("p_mT", 32)
            p_kt = pool("p_kt", 16)
            p_vt = pool("p_vt", 32)
            p_E = pool("p_E", 8)
            p_w = pool("p_w", 10)
            p_w2 = pool("p_w2", 6)
            p_f32 = pool("p_f32", 9)
            p_row = pool("p_row", 6)
            p_rowr = pool("p_rowr", 2)
            p_lmbbc = pool("p_lmbbc", 8)
            p_lnp = pool("p_lnp", 2)
            p_sacc = pool("p_sacc", 32)
            p_const = pool("p_const", 1)
            ps_mm = pool("ps_mm", 4, "PSUM")
            ps_acc = pool("ps_acc", 2, "PSUM")
            ps_row = pool("ps_row", 2, "PSUM")
            # ---- constants ----
            tri_sb = p_const.tile([128, 256], F32, tag="tri")
            nc.sync.dma_start(tri_sb[:], tri2[:])
            maska_sb = p_const.tile([128, 1024], F32, tag="ma")
            nc.sync.dma_start(maska_sb[:], maska[:])
            maskbt_sb = p_const.tile([128, 1024], F32, tag="mbt")
            nc.sync.dma_start(maskbt_sb[:], maskbt[:])
            lnfp_sb = p_const.tile([128, 16], F32, tag="lnfp")
            nc.sync.dma_start(lnfp_sb[:], lnfp[:])

            ones_f32 = p_const.tile([128, 128], F32, tag="o32")
            nc.vector.memset(ones_f32[:], 1.0)
            ones_bf = p_const.tile([128, 32], BF16, tag="obf")
            nc.vector.memset(ones_bf[:], 1.0)
            ones64r = p_const.tile([1, 64], F32R, tag="o64r")
            nc.vector.tensor_copy(ones64r[:], ones_f32[0:1, 0:64])
            ones128r = p_const.tile([1, 128], F32R, tag="o128r")
            nc.vector.tensor_copy(ones128r[:], ones_f32[0:1, :])
            eps_sb = p_const.tile([1, 1], F32, tag="eps")
            nc.vector.memset(eps_sb[:], EPS)

            # v-ones columns in kv_send (written once; col 64/129 of each
            # [128,130] v block stays 1.0 across layers)
            vpart = kv_send[VOFF:NRANK].rearrange(
                "(s sl hp p c) -> s sl hp p c", s=2, sl=2, hp=8, p=128, c=130)
            for s in range(2):
                for sl in range(2):
                    for hp in range(8):
                        nc.sync.dma_start(vpart[s, sl, hp, :, 64:65],
                                          ones_bf[:, 0:1])
                        nc.sync.dma_start(vpart[s, sl, hp, :, 129:130],
                                          ones_bf[:, 1:2])

            # ---- residual stream x^T (f32, persists) ----
            x_tiles = []
            for k in range(KT):
                xt = p_x.tile([128, TOK], F32, tag="x", name=f"x_{k}")
                nc.sync.dma_start(xt[:], x0t[k])
                x_tiles.append(xt)

            def layernorm(wcol, bcol, name):
                """LN over E of x_tiles -> list of bf16 [128,TOK] tiles."""
                xb = []
                for k in range(KT):
                    t = p_ln.tile([128, TOK], BF16, tag="t", name=f"{name}xb{k}")
                    nc.vector.tensor_copy(t[:], x_tiles[k][:])
                    xb.append(t)
                sq = []
                for k in range(KT):
                    t = p_ln.tile([128, TOK], BF16, tag="t", name=f"{name}sq{k}")
                    nc.scalar.activation(t[:], xb[k][:], SQUARE)
                    sq.append(t)
                ps_s = ps_row.tile([1, TOK], F32, tag="r", name=f"{name}ps_s")
                ps_q = ps_row.tile([1, TOK], F32, tag="r", name=f"{name}ps_q")
                for k in range(KT):
                    nc.tensor.matmul(ps_s[:], ones_bf[:, 0:1], xb[k][:],
                                     start=(k == 0), stop=(k == KT - 1))
                for k in range(KT):
                    nc.tensor.matmul(ps_q[:], ones_bf[:, 0:1], sq[k][:],
                                     start=(k == 0), stop=(k == KT - 1))
                mean = p_row.tile([1, TOK], F32, tag="row", name=f"{name}mean")
                nc.scalar.mul(mean[:], ps_s[:], 1.0 / E)
                ex2 = p_row.tile([1, TOK], F32, tag="row", name=f"{name}ex2")
                nc.scalar.mul(ex2[:], ps_q[:], 1.0 / E)
                msq = p_row.tile([1, TOK], F32, tag="row", name=f"{name}msq")
                nc.scalar.activation(msq[:], mean[:], SQUARE)
                var = p_row.tile([1, TOK], F32, tag="row", name=f"{name}var")
                nc.vector.tensor_sub(var[:], ex2[:], msq[:])
                std = p_row.tile([1, TOK], F32, tag="row", name=f"{name}std")
                nc.scalar.activation(std[:], var[:], SQRT, bias=eps_sb[:])
                rstd = p_row.tile([1, TOK], F32, tag="row", name=f"{name}rstd")
                nc.vector.reciprocal(rstd[:], std[:])
                # broadcast mean/rstd across partitions via K=1 f32r matmuls
                meanr = p_rowr.tile([1, TOK], F32R, tag="rr", name=f"{name}meanr")
                nc.vector.tensor_copy(meanr[:], mean[:])
                rstdr = p_rowr.tile([1, TOK], F32R, tag="rr", name=f"{name}rstdr")
                nc.vector.tensor_copy(rstdr[:], rstd[:])
                mb = ps_mm.tile([128, TOK], F32, tag="mm", name=f"{name}mb")
                nc.tensor.matmul(mb[:], ones128r[:], meanr[:], start=True,
                                 stop=True)
                rb = ps_mm.tile([128, TOK], F32, tag="mm", name=f"{name}rb")
                nc.tensor.matmul(rb[:], ones128r[:], rstdr[:], start=True,
                                 stop=True)
                out = []
                for k in range(KT):
                    t1 = p_f32.tile([128, TOK], F32, tag="f", name=f"{name}t1_{k}")
                    nc.vector.tensor_sub(t1[:], x_tiles[k][:], mb[:])
                    t2 = p_f32.tile([128, TOK], F32, tag="f", name=f"{name}t2_{k}")
                    nc.vector.tensor_mul(t2[:], t1[:], rb[:])
                    h = p_h.tile([128, TOK], BF16, tag="h", name=f"{name}h{k}")
                    nc.scalar.activation(h[:], t2[:], IDENT,
                                         bias=bcol[k], scale=wcol[k])
                    out.append(h)
                return out

            for layer in range(n_layers):
                lp = p_lnp.tile([128, 80], F32, tag="lnp", name=f"lnp{layer}")
                nc.sync.dma_start(lp[:], lnp[layer])

                h1 = layernorm([lp[:, C_LN1W + k:C_LN1W + k + 1] for k in range(KT)],
                               [lp[:, C_LN1B + k:C_LN1B + k + 1] for k in range(KT)],
                               f"L{layer}ln1")

                # ---- K, V projections first (feed the AllGather early) ----
                wk_sb = []
                for k in range(KT):
                    t = p_w.tile([128, E], BF16, tag="w", name=f"L{layer}wk{k}")
                    nc.sync.dma_start(t[:], wk[layer, k * 128:(k + 1) * 128, :])
                    wk_sb.append(t)
                kT_sb = []
                for hp in range(8):
                    ps = ps_mm.tile([128, TOK], F32, tag="mm",
                                    name=f"L{layer}psk{hp}")
                    for k in range(KT):
                        nc.tensor.matmul(ps[:], wk_sb[k][:, hp * 128:(hp + 1) * 128],
                                         h1[k][:], start=(k == 0),
                                         stop=(k == KT - 1))
                    t = p_qk.tile([128, TOK], BF16, tag="qk",
                                  name=f"L{layer}kT{hp}")
                    nc.vector.tensor_copy(t[:], ps[:])
                    kT_sb.append(t)
                    for s in range(2):
                        nc.sync.dma_start(
                            kv_send[_koff(s, hp):_koff(s, hp) + 32768]
                            .rearrange("(p c) -> p c", p=128),
                            t[:, s * 256:(s + 1) * 256])

                wv_sb = []
                for k in range(KT):
                    t = p_w.tile([128, E], BF16, tag="w", name=f"L{layer}wv{k}")
                    nc.sync.dma_start(t[:], wv[layer, k * 128:(k + 1) * 128, :])
                    wv_sb.append(t)
                vloc = {}
                for s in range(2):
                    for sl in range(2):
                        tc_idx = s * 2 + sl  # token chunk index in TOK cols
                        for g in range(4):  # head groups of 4
                            ps = ps_mm.tile([128, 256], F32, tag="mm",
                                            name=f"L{layer}psv{tc_idx}_{g}")
                            for k in range(KT):
                                nc.tensor.matmul(
                                    ps[:],
                                    h1[k][:, tc_idx * 128:(tc_idx + 1) * 128],
                                    wv_sb[k][:, g * 256:(g + 1) * 256],
                                    start=(k == 0), stop=(k == KT - 1))
                            for hh in range(2):  # head pairs within group
                                hp = g * 2 + hh
                                vv = p_vloc.tile([128, 130], BF16, tag="vl",
                                                 name=f"L{layer}vv{s}{sl}{hp}")
                                nc.vector.tensor_copy(
                                    vv[:].rearrange("p (h c) -> p h c", h=2)
                                    [:, :, 0:64],
                                    ps[:, hh * 128:(hh + 1) * 128]
                                    .rearrange("p (h c) -> p h c", h=2))
                                nc.vector.memset(
                                    vv[:].rearrange("p (h c) -> p h c", h=2)
                                    [:, :, 64:65], 1.0)
                                vloc[(s, sl, hp)] = vv
                                nc.sync.dma_start(
                                    kv_send[_voff(s, sl, hp):
                                            _voff(s, sl, hp) + 16640]
                                    .rearrange("(p c) -> p c", p=128)
                                    [:, 0:130].rearrange("p (h c) -> p h c", h=2)
                                    [:, :, 0:64],
                                    vv[:].rearrange("p (h c) -> p h c", h=2)
                                    [:, :, 0:64])

                nc.gpsimd.collective_compute(
                    "AllGather", mybir.AluOpType.bypass,
                    replica_groups=[list(range(NCORE))],
                    ins=[kv_send[:].opt()], outs=[kv_gath[:].opt()])

                # ---- Q projection (overlaps the AllGather) ----
                wq_sb = []
                for k in range(KT):
                    t = p_w.tile([128, E], BF16, tag="w", name=f"L{layer}wq{k}")
                    nc.sync.dma_start(t[:], wq[layer, k * 128:(k + 1) * 128, :])
                    wq_sb.append(t)
                qT_sb = []
                for hp in range(8):
                    ps = ps_mm.tile([128, TOK], F32, tag="mm",
                                    name=f"L{layer}psq{hp}")
                    for k in range(KT):
                        nc.tensor.matmul(ps[:], wq_sb[k][:, hp * 128:(hp + 1) * 128],
                                         h1[k][:], start=(k == 0),
                                         stop=(k == KT - 1))
                    t = p_qk.tile([128, TOK], BF16, tag="qk",
                                  name=f"L{layer}qT{hp}")
                    nc.vector.tensor_copy(t[:], ps[:])
                    qT_sb.append(t)

                # ---- attention ----
                oT_sb = []
                for k in range(KT):
                    t = p_oT.tile([128, TOK], BF16, tag="o", name=f"L{layer}oT{k}")
                    oT_sb.append(t)

                for s in range(2):
                    for hp in range(8):
                        kt_t = []
                        for r in range(NCORE):
                            t = p_kt.tile([128, 256], BF16, tag="kt",
                                          name=f"L{layer}kt{s}{hp}_{r}")
                            off = r * NRANK + _koff(s, hp)
                            nc.sync.dma_start(
                                t[:], kv_gath[off:off + 32768]
                                .rearrange("(p c) -> p c", p=128))
                            kt_t.append(t)
                        vt_t = []
                        for j in range(16):
                            r, sl = (j, 0) if j < 8 else (15 - j, 1)
                            t = p_vt.tile([128, 130], BF16, tag="vt",
                                          name=f"L{layer}vt{s}{hp}_{j}")
                            off = r * NRANK + _voff(s, sl, hp)
                            nc.sync.dma_start(
                                t[:], kv_gath[off:off + 16640]
                                .rearrange("(p c) -> p c", p=128))
                            vt_t.append(t)

                        def kslice(j):
                            if j < 8:
                                return kt_t[j], 0
                            return kt_t[15 - j], 128

                        for hh in range(2):
                            ro = hh * 64
                            qa = qT_sb[hp][ro:ro + 64, s * 256:s * 256 + 128]
                            qb = qT_sb[hp][ro:ro + 64, s * 256 + 128:s * 256 + 256]
                            tp = (ro, 0)

                            def scores4(js, qcols, name):
                                ps = ps_mm.tile([128, 512], F32, tag="mm",
                                                name=name)
                                for i, j in enumerate(js):
                                    kt_tile, co = kslice(j)
                                    nc.tensor.matmul(
                                        ps[:, i * 128:(i + 1) * 128],
                                        kt_tile[ro:ro + 64, co:co + 128],
                                        qcols, start=True, stop=True,
                                        tile_position=tp)
                                return ps

                            nm = f"L{layer}a{s}{hp}{hh}"
                            e_tiles = {}
                            # A groups (chunk a, kv j=0..7, bias-masked)
                            for gi in range(2):
                                js = list(range(gi * 4, gi * 4 + 4))
                                ps = scores4(js, qa, f"{nm}psA{gi}")
                                tmp = p_f32.tile([128, 512], F32, tag="f",
                                                 name=f"{nm}tmA{gi}")
                                nc.vector.tensor_tensor(
                                    tmp[:], ps[:],
                                    maska_sb[:, gi * 512:(gi + 1) * 512], op=ADD)
                                et = p_E.tile([128, 512], BF16, tag="E",
                                              name=f"{nm}eA{gi}")
                                nc.scalar.activation(et[:], tmp[:], EXPF,
                                                     scale=0.125)
                                for i, j in enumerate(js):
                                    e_tiles[("a", j)] = (et, i * 128)
                            # B head (chunk b, kv j=0..7, never masked)
                            for gi in range(2):
                                js = list(range(gi * 4, gi * 4 + 4))
                                ps = scores4(js, qb, f"{nm}psBh{gi}")
                                et = p_E.tile([128, 512], BF16, tag="E",
                                              name=f"{nm}eBh{gi}")
                                nc.scalar.activation(et[:], ps[:], EXPF,
                                                     scale=0.125)
                                for i, j in enumerate(js):
                                    e_tiles[("b", j)] = (et, i * 128)
                            # B tail (chunk b, kv j=8..15, bias-masked)
                            for gi in range(2):
                                js = list(range(8 + gi * 4, 8 + gi * 4 + 4))
                                ps = scores4(js, qb, f"{nm}psBt{gi}")
                                tmp = p_f32.tile([128, 512], F32, tag="f",
                                                 name=f"{nm}tmBt{gi}")
                                nc.vector.tensor_tensor(
                                    tmp[:], ps[:],
                                    maskbt_sb[:, gi * 512:(gi + 1) * 512], op=ADD)
                                et = p_E.tile([128, 512], BF16, tag="E",
                                              name=f"{nm}eBt{gi}")
                                nc.scalar.activation(et[:], tmp[:], EXPF,
                                                     scale=0.125)
                                for i, j in enumerate(js):
                                    e_tiles[("b", j)] = (et, i * 128)
                            # diagonal blocks from LOCAL k/q (chunk a & b)
                            psd = ps_mm.tile([128, 256], F32, tag="mm",
                                             name=f"{nm}psD")
                            nc.tensor.matmul(
                                psd[:, 0:128],
                                kT_sb[hp][ro:ro + 64, s * 256:s * 256 + 128],
                                qa, start=True, stop=True, tile_position=tp)
                            nc.tensor.matmul(
                                psd[:, 128:256],
                                kT_sb[hp][ro:ro + 64,
                                          s * 256 + 128:s * 256 + 256],
                                qb, start=True, stop=True, tile_position=tp)
                            tmpd = p_f32.tile([128, 256], F32, tag="f",
                                              name=f"{nm}tmD")
                            nc.vector.tensor_tensor(tmpd[:], psd[:], tri_sb[:],
                                                    op=ADD)
                            etd = p_E.tile([128, 256], BF16, tag="E",
                                           name=f"{nm}eD")
                            nc.scalar.activation(etd[:], tmpd[:], EXPF,
                                                 scale=0.125)

                            # o accumulation [65, 256]: cols 0:128 chunk a,
                            # 128:256 chunk b
                            po = ps_acc.tile([65, 256], F32, tag="acc",
                                             name=f"{nm}po")
                            for i, j in enumerate(range(8)):  # chunk a
                                et, eo = e_tiles[("a", j)]
                                nc.tensor.matmul(
                                    po[:, 0:128],
                                    vt_t[j][:, hh * 65:hh * 65 + 65],
                                    et[:, eo:eo + 128],
                                    start=(i == 0), stop=False)
                            nc.tensor.matmul(
                                po[:, 0:128],
                                vloc[(s, 0, hp)][:, hh * 65:hh * 65 + 65],
                                etd[:, 0:128], start=False, stop=True)
                            for i, j in enumerate(range(16)):  # chunk b
                                et, eo = e_tiles[("b", j)]
                                nc.tensor.matmul(
                                    po[:, 128:256],
                                    vt_t[j][:, hh * 65:hh * 65 + 65],
                                    et[:, eo:eo + 128],
                                    start=(i == 0), stop=False)
                            nc.tensor.matmul(
                                po[:, 128:256],
                                vloc[(s, 1, hp)][:, hh * 65:hh * 65 + 65],
                                etd[:, 128:256], start=False, stop=True)

                            # normalize: o / sumexp (row 64)
                            rrow = p_row.tile([1, 256], F32, tag="row",
                                              name=f"{nm}rrow")
                            nc.vector.reciprocal(rrow[:], po[64:65, :])
                            rrr = p_rowr.tile([1, 256], F32R, tag="rr",
                                              name=f"{nm}rrr")
                            nc.vector.tensor_copy(rrr[:], rrow[:])
                            prb = ps_row.tile([64, 256], F32, tag="r",
                                              name=f"{nm}prb")
                            nc.tensor.matmul(prb[:], ones64r[:], rrr[:],
                                             start=True, stop=True)
                            rbs = p_f32.tile([64, 256], F32, tag="f",
                                             name=f"{nm}rbs")
                            nc.scalar.copy(rbs[:], prb[:])
                            h = hp * 2 + hh
                            dst = oT_sb[h // 2][(h % 2) * 64:(h % 2) * 64 + 64,
                                                s * 256:s * 256 + 256]
                            nc.vector.tensor_mul(dst, po[0:64, :], rbs[:])

                # ---- output projection + residual ----
                wo_sb = []
                for k in range(KT):
                    t = p_w.tile([128, E], BF16, tag="w", name=f"L{layer}wo{k}")
                    nc.sync.dma_start(t[:], wo[layer, k * 128:(k + 1) * 128, :])
                    wo_sb.append(t)
                for eo in range(KT):
                    ps = ps_mm.tile([128, TOK], F32, tag="mm",
                                    name=f"L{layer}psd{eo}")
                    for k in range(KT):
                        nc.tensor.matmul(ps[:],
                                         wo_sb[k][:, eo * 128:(eo + 1) * 128],
                                         oT_sb[k][:], start=(k == 0),
                                         stop=(k == KT - 1))
                    ds = p_f32.tile([128, TOK], F32, tag="f",
                                    name=f"L{layer}ds{eo}")
                    nc.scalar.activation(ds[:], ps[:], IDENT,
                                         bias=lp[:, C_BO + eo:C_BO + eo + 1])
                    nc.vector.tensor_add(x_tiles[eo][:], x_tiles[eo][:], ds[:])

                # ---- FFN ----
                h2 = layernorm([lp[:, C_LN2W + k:C_LN2W + k + 1] for k in range(KT)],
                               [lp[:, C_LN2B + k:C_LN2B + k + 1] for k in range(KT)],
                               f"L{layer}ln2")
                m_sb = []
                for q in range(4):
                    w1_sb = []
                    for k in range(KT):
                        t = p_w.tile([128, E], BF16, tag="w",
                                     name=f"L{layer}w1_{q}_{k}")
                        nc.sync.dma_start(
                            t[:], w1[layer, k * 128:(k + 1) * 128,
                                     q * 1024:(q + 1) * 1024])
                        w1_sb.append(t)
                    for mo in range(8):
                        mi = q * 8 + mo
                        ps = ps_mm.tile([128, TOK], F32, tag="mm",
                                        name=f"L{layer}psm{mi}")
                        for k in range(KT):
                            nc.tensor.matmul(
                                ps[:], w1_sb[k][:, mo * 128:(mo + 1) * 128],
                                h2[k][:], start=(k == 0), stop=(k == KT - 1))
                        mt = p_mT.tile([128, TOK], BF16, tag="m",
                                       name=f"L{layer}m{mi}")
                        nc.scalar.activation(
                            mt[:], ps[:], RELU,
                            bias=lp[:, C_B1 + mi:C_B1 + mi + 1])
                        m_sb.append(mt)
                for eop in range(4):
                    pse = [ps_mm.tile([128, TOK], F32, tag="mm",
                                      name=f"L{layer}pse{eop}_{u}")
                           for u in range(2)]
                    for k in range(32):
                        t = p_w2.tile([128, 256], BF16, tag="w2",
                                      name=f"L{layer}w2_{eop}_{k}")
                        nc.sync.dma_start(
                            t[:], w2[layer, k * 128:(k + 1) * 128,
                                     eop * 256:(eop + 1) * 256])
                        for u in range(2):
                            nc.tensor.matmul(pse[u][:],
                                             t[:, u * 128:(u + 1) * 128],
                                             m_sb[k][:], start=(k == 0),
                                             stop=(k == 31))
                    for u in range(2):
                        eo = eop * 2 + u
                        ds = p_f32.tile([128, TOK], F32, tag="f",
                                        name=f"L{layer}ds2{eo}")
                        nc.scalar.activation(
                            ds[:], pse[u][:], IDENT,
                            bias=lp[:, C_B2 + eo:C_B2 + eo + 1])
                        nc.vector.tensor_add(x_tiles[eo][:], x_tiles[eo][:],
                                             ds[:])

            if debug_xt:
                for k in range(KT):
                    nc.sync.dma_start(dbg_o[k], x_tiles[k][:])

            # ---- final LN + AllGather of xf^T ----
            xf = layernorm([lnfp_sb[:, k:k + 1] for k in range(KT)],
                           [lnfp_sb[:, 8 + k:8 + k + 1] for k in range(KT)],
                           "lnf")
            for k in range(KT):
                nc.sync.dma_start(
                    xf_send[k * 128 * TOK:(k + 1) * 128 * TOK]
                    .rearrange("(p c) -> p c", p=128), xf[k][:])
            nc.gpsimd.collective_compute(
                "AllGather", mybir.AluOpType.bypass,
                replica_groups=[list(range(NCORE))],
                ins=[xf_send[:].opt()], outs=[xf_gath[:].opt()])

            # ---- lm_b broadcast tiles ----
            lmb_bc = []
            for n in range(8):
                row = p_row.tile([1, 500], F32, tag="row", name=f"lmbrow{n}")
                nc.sync.dma_start(row[:], lmb[n:n + 1, :])
                rowr = p_rowr.tile([1, 500], F32R, tag="rr", name=f"lmbr{n}")
                nc.vector.tensor_copy(rowr[:], row[:])
                pb = ps_mm.tile([128, 500], F32, tag="mm", name=f"lmbps{n}")
                nc.tensor.matmul(pb[:], ones128r[:], rowr[:], start=True,
                                 stop=True)
                t = p_lmbbc.tile([128, 500], BF16, tag="bb", name=f"lmbbc{n}")
                nc.vector.tensor_copy(t[:], pb[:])
                lmb_bc.append(t)

            # ---- LM head: logits[tok, vocab-shard] ----
            sacc = [p_sacc.tile([128, 8], F32, tag="sa", name=f"sacc{m}")
                    for m in range(32)]
            for n in range(8):
                lw_sb = []
                for k in range(KT):
                    t = p_w.tile([128, 500], BF16, tag="w", name=f"lw{n}_{k}")
                    nc.sync.dma_start(
                        t[:], lmw[k * 128:(k + 1) * 128,
                                  n * 500:(n + 1) * 500])
                    lw_sb.append(t)
                for r in range(NCORE):
                    xfg = []
                    for k in range(KT):
                        t = p_mT.tile([128, TOK], BF16, tag="m",
                                      name=f"xfg{n}_{r}_{k}")
                        off = (r * KT + k) * 128 * TOK
                        nc.sync.dma_start(
                            t[:], xf_gath[off:off + 128 * TOK]
                            .rearrange("(p c) -> p c", p=128))
                        xfg.append(t)
                    for ml in range(4):
                        m = r * 4 + ml
                        ps = ps_mm.tile([128, 500], F32, tag="mm",
                                        name=f"lg{n}_{m}")
                        for k in range(KT):
                            nc.tensor.matmul(
                                ps[:], xfg[k][:, ml * 128:(ml + 1) * 128],
                                lw_sb[k][:], start=(k == 0),
                                stop=(k == KT - 1))
                        lg = p_f32.tile([128, 500], F32, tag="f",
                                        name=f"lgs{n}_{m}")
                        nc.vector.tensor_tensor(lg[:], ps[:], lmb_bc[n][:],
                                                op=ADD)
                        nc.sync.dma_start(
                            logits_o[m * 128:(m + 1) * 128,
                                     n * 500:(n + 1) * 500], lg[:])
                        # exp row-sums for the loss
                        junk = p_f32.tile([128, 500], F32, tag="f",
                                          name=f"ej{n}_{m}")
                        nc.scalar.activation(junk[:], lg[:], EXPF,
                                             accum_out=sacc[m][:, n:n + 1])
            # assemble sexp: for each m, sum the 8 per-shard-chunk sums
            for m in range(32):
                srow = p_row.tile([128, 1], F32, tag="row", name=f"srow{m}")
                nc.vector.reduce_sum(srow[:], sacc[m][:],
                                     axis=mybir.AxisListType.X)
                nc.sync.dma_start(sexp_o[m:m + 1, :].rearrange("a b -> (a b)"),
                                  srow[:])
    nc.compile()
    return nc


_NC_CACHE = {}


def _get_nc(n_layers=L, debug_xt=False):
    key = (n_layers, debug_xt)
    if key not in _NC_CACHE:
        _NC_CACHE[key] = build(n_layers, debug_xt)
    return _NC_CACHE[key]


def _pos_encoding(t, e):
    pos = np.arange(t, dtype=np.float32)[:, None]
    div = np.exp(np.arange(0, e, 2, dtype=np.float32)
                 * np.float32(-math.log(10000.0) / e))
    ang = pos * div
    return np.stack([np.sin(ang), np.cos(ang)], axis=-1).reshape(t, e) \
        .astype(np.float32)


def _chunks_of(r):
    return (r, NCH - 1 - r)


def _token_perm():
    """gathered row g -> flat (b*T + t) index."""
    perm = np.empty(NCORE * TOK, dtype=np.int64)
    g = 0
    for r in range(NCORE):
        ca, cb = _chunks_of(r)
        for s in range(B):
            for c in (ca, cb):
                for p in range(128):
                    perm[g] = s * T + c * 128 + p
                    g += 1
    return perm


def _host_inputs(inputs, n_layers=L):
    bf = ml_dtypes.bfloat16
    tok_emb = np.asarray(inputs["tok_emb"], np.float32)
    idx = np.asarray(inputs["idx"])
    x0 = tok_emb[idx] + _pos_encoding(T, E)[None, :, :]   # [B, T, E] f32

    shared = {
        "wq": np.ascontiguousarray(np.asarray(inputs["wq"], np.float32)).astype(bf),
        "wk": np.ascontiguousarray(np.asarray(inputs["wk"], np.float32)).astype(bf),
        "wv": np.ascontiguousarray(np.asarray(inputs["wv"], np.float32)).astype(bf),
        "wo": np.ascontiguousarray(np.asarray(inputs["wo"], np.float32)).astype(bf),
        "w1": np.ascontiguousarray(np.asarray(inputs["w1"], np.float32)).astype(bf),
        "w2": np.ascontiguousarray(np.asarray(inputs["w2"], np.float32)).astype(bf),
    }
    lnp = np.zeros((L, 128, 80), np.float32)
    for l in range(L):
        for base, nm in ((C_LN1W, "ln1_w"), (C_LN1B, "ln1_b"),
                         (C_LN2W, "ln2_w"), (C_LN2B, "ln2_b"),
                         (C_BO, "bo"), (C_B2, "b2")):
            v = np.asarray(inputs[nm], np.float32)[l]          # [1024]
            lnp[l, :, base:base + 8] = v.reshape(8, 128).T
        v = np.asarray(inputs["b1"], np.float32)[l]            # [4096]
        lnp[l, :, C_B1:C_B1 + 32] = v.reshape(32, 128).T
    shared["lnp"] = lnp
    lnfp = np.zeros((128, 16), np.float32)
    lnfp[:, 0:8] = np.asarray(inputs["lnf_w"], np.float32).reshape(8, 128).T
    lnfp[:, 8:16] = np.asarray(inputs["lnf_b"], np.float32).reshape(8, 128).T
    shared["lnfp"] = lnfp
    tri = np.where(np.arange(128)[:, None] <= np.arange(128)[None, :],
                   np.float32(0.0), np.float32(NEG))
    shared["tri2"] = np.concatenate([tri, tri], axis=1)

    lm_w = np.ascontiguousarray(np.asarray(inputs["lm_w"], np.float32)).astype(bf)
    lm_b = np.asarray(inputs["lm_b"], np.float32)

    in_maps = []
    for r in range(NCORE):
        ca, cb = _chunks_of(r)
        cols = []
        for s in range(B):
            for c in (ca, cb):
                cols.append(x0[s, c * 128:(c + 1) * 128, :])   # [128, E]
        xr = np.concatenate(cols, axis=0).T                    # [E, TOK]
        x0t = np.ascontiguousarray(xr.reshape(KT, 128, TOK))

        maska = np.zeros((128, 1024), np.float32)
        maskbt = np.zeros((128, 1024), np.float32)
        for j in range(8):
            if j >= ca:                       # chunk a diag/future -> masked
                maska[:, j * 128:(j + 1) * 128] = NEG
        for j in range(8, 16):
            if j >= cb:                       # chunk b diag/future -> masked
                maskbt[:, (j - 8) * 128:(j - 7) * 128] = NEG

        m = dict(shared)
        m["x0t"] = x0t
        m["maska"] = maska
        m["maskbt"] = maskbt
        m["lmw"] = np.ascontiguousarray(lm_w[:, r * VS:(r + 1) * VS])
        m["lmb"] = np.ascontiguousarray(
            lm_b[r * VS:(r + 1) * VS].reshape(8, 500))
        in_maps.append(m)
    return in_maps


def _assemble(results, inputs):
    perm = _token_perm()
    logits = np.empty((B * T, V), np.float32)
    sexp_g = np.zeros(NCORE * TOK, np.float64)
    for r in range(NCORE):
        logits[perm, r * VS:(r + 1) * VS] = results[r]["logits_s"]
        sexp_g += results[r]["sexp"].reshape(-1).astype(np.float64)
    lse_g = np.log(sexp_g)                                  # gathered order
    lse = np.empty(B * T, np.float64)
    lse[perm] = lse_g
    targets = np.asarray(inputs["targets"]).reshape(-1)
    tgt_logit = logits[np.arange(B * T), targets].astype(np.float64)
    loss = np.float32(np.mean(lse - tgt_logit))
    return logits, loss


def kernel(**inputs):
    nc = _get_nc()
    in_maps = _host_inputs(inputs)
    res = run_bass_kernel_spmd(nc, in_maps, list(range(NCORE)))
    return _assemble(res.results, inputs)


# revision 26
# speedup vs baseline: 1.0365x; 1.0365x over previous
"""Self-contained Trainium2 Bass kernel for the GPT forward pass.

B=2, T=2048, V=32000, E=1024, H=16, HS=64, L=8 -> (logits [4096,32000], loss).

Strategy: sequence-data-parallel over 8 NeuronCores. Each core owns 512
tokens (zigzag chunks r and 15-r of each sample for causal balance) and
holds a full replica of the weights (pre-cast to bf16 on host). Per layer
one AllGather ships K/V (bf16). The LM head is vocab-sharded (4000
columns per core) after an AllGather of the final-LN activations.
Activations are kept transposed ([E, tok]) in SBUF; LayerNorm statistics
use ones-matmuls over the partition dim; softmax uses exp without max
subtraction (scores are bounded because inputs are LayerNormed); causal
masks arrive as per-core host data so the SPMD graph is identical on all
cores; a ones column appended to V produces the softmax denominator for
free.
"""

import math
import sys

import numpy as np
import ml_dtypes

sys.path.insert(0, "/opt/trn_rl_repo")

import concourse.bass as bass  # noqa: E402
import concourse.mybir as mybir  # noqa: E402
from concourse import bacc, tile  # noqa: E402
from concourse.bass_utils import run_bass_kernel_spmd  # noqa: E402

F32 = mybir.dt.float32
F32R = mybir.dt.float32r
BF16 = mybir.dt.bfloat16
EXPF = mybir.ActivationFunctionType.Exp
IDENT = mybir.ActivationFunctionType.Identity
RELU = mybir.ActivationFunctionType.Relu
SQUARE = mybir.ActivationFunctionType.Square
SQRT = mybir.ActivationFunctionType.Sqrt
ADD = mybir.AluOpType.add

B, T, V, E, H, L = 2, 2048, 32000, 1024, 16, 8
HS = E // H          # 64
NCORE = 8
TOK = 512            # tokens per core: 2 samples x 2 chunks x 128
VS = V // NCORE      # 4000 vocab per core
NCH = T // 128       # 16 chunks per sample
KT = E // 128        # 8 k-tiles over E
EPS = 1e-5
NEG = -1.0e30

# kv_send layout (elements, bf16):
#   k part: [s(2)][hp(8)][128 x 256]            -> 2*8*32768   = 524288
#   v part: [s(2)][slot(2)][hp(8)][128 x 130]   -> 2*2*8*16640 = 532480
KOFF = 0
VOFF = 2 * 8 * 32768
NRANK = VOFF + 2 * 2 * 8 * 16640  # 1056768 elems per rank


def _koff(s, hp):
    return KOFF + s * 8 * 32768 + hp * 32768


def _voff(s, slot, hp):
    return VOFF + s * (2 * 8 * 16640) + slot * (8 * 16640) + hp * 16640


# lnp packed param columns (f32, [L, 128, 80]):
C_LN1W, C_LN1B, C_LN2W, C_LN2B, C_BO, C_B2, C_B1 = 0, 8, 16, 24, 32, 40, 48


def build(n_layers=L, debug_xt=False):
    nc = bacc.Bacc(None, num_devices=NCORE)

    x0t = nc.declare_dram_parameter("x0t", [KT, 128, TOK], F32, isOutput=False)
    wq = nc.declare_dram_parameter("wq", [L, E, E], BF16, isOutput=False)
    wk = nc.declare_dram_parameter("wk", [L, E, E], BF16, isOutput=False)
    wv = nc.declare_dram_parameter("wv", [L, E, E], BF16, isOutput=False)
    wo = nc.declare_dram_parameter("wo", [L, E, E], BF16, isOutput=False)
    w1 = nc.declare_dram_parameter("w1", [L, E, 4 * E], BF16, isOutput=False)
    w2 = nc.declare_dram_parameter("w2", [L, 4 * E, E], BF16, isOutput=False)
    lnp = nc.declare_dram_parameter("lnp", [L, 128, 80], F32, isOutput=False)
    lnfp = nc.declare_dram_parameter("lnfp", [128, 16], F32, isOutput=False)
    lmw = nc.declare_dram_parameter("lmw", [E, VS], BF16, isOutput=False)
    lmb = nc.declare_dram_parameter("lmb", [8, 500], F32, isOutput=False)
    maskab = nc.declare_dram_parameter("maskab", [128, 2048], BF16,
                                       isOutput=False)
    maskbt = nc.declare_dram_parameter("maskbt", [128, 1024], BF16,
                                       isOutput=False)
    tri2 = nc.declare_dram_parameter("tri2", [128, 256], BF16, isOutput=False)

    logits_o = nc.declare_dram_parameter("logits_s", [NCORE * TOK, VS], F32,
                                         isOutput=True)
    sexp_o = nc.declare_dram_parameter("sexp", [32, 128], F32, isOutput=True)
    if debug_xt:
        dbg_o = nc.declare_dram_parameter("dbg_xt", [KT, 128, TOK], F32,
                                          isOutput=True)

    kv_send = nc.dram_tensor("kv_send", [NRANK], BF16)
    kv_gath = nc.dram_tensor("kv_gath", [NCORE * NRANK], BF16,
                             addr_space="Shared")
    xf_send = nc.dram_tensor("xf_send", [KT * 128 * TOK], BF16)
    xf_gath = nc.dram_tensor("xf_gath", [NCORE * KT * 128 * TOK], BF16,
                             addr_space="Shared")

    from contextlib import ExitStack
    with tile.TileContext(nc) as tc:
        with ExitStack() as ctx:
            def pool(name, bufs, space="SBUF"):
                return ctx.enter_context(
                    tc.tile_pool(name=name, bufs=bufs, space=space))
            p_x = pool("p_x", 8)
            p_ln = pool("p_ln", 6)
            p_h = pool("p_h", 8)
            p_qk = pool("p_qk", 16)
            p_vloc = pool("p_vloc", 32)
            p_oT = pool("p_oT", 8)
            p_mT = pool("p_mT", 32)
            p_kt = pool("p_kt", 2)
            p_vt = pool("p_vt", 4)
            p_E = pool("p_E", 7)
            p_w = pool("p_w", 8)
            p_w2 = pool("p_w2", 2)
            p_xfa = pool("p_xfa", 2)
            p_f32 = pool("p_f32", 8)
            p_row = pool("p_row", 5)
            p_rowr = pool("p_rowr", 2)
            p_lmbbc = pool("p_lmbbc", 8)
            p_lnp = pool("p_lnp", 2)
            p_sacc = pool("p_sacc", 32)
            p_const = pool("p_const", 1)
            ps_mm = pool("ps_mm", 4, "PSUM")
            ps_acc = pool("ps_acc", 2, "PSUM")
            ps_row = pool("ps_row", 2, "PSUM")
            # ---- constants ----
            tri_sb = p_const.tile([128, 256], BF16, tag="tri")
            nc.sync.dma_start(tri_sb[:], tri2[:])
            maskab_sb = p_const.tile([128, 2048], BF16, tag="ma")
            nc.sync.dma_start(maskab_sb[:], maskab[:])
            maskbt_sb = p_const.tile([128, 1024], BF16, tag="mbt")
            nc.sync.dma_start(maskbt_sb[:], maskbt[:])
            lnfp_sb = p_const.tile([128, 16], F32, tag="lnfp")
            nc.sync.dma_start(lnfp_sb[:], lnfp[:])

            ones_f32 = p_const.tile([128, 128], F32, tag="o32")
            nc.vector.memset(ones_f32[:], 1.0)
            ones_bf = p_const.tile([128, 32], BF16, tag="obf")
            nc.vector.memset(ones_bf[:], 1.0)
            ones64r = p_const.tile([1, 64], F32R, tag="o64r")
            nc.vector.tensor_copy(ones64r[:], ones_f32[0:1, 0:64])
            ones128r = p_const.tile([1, 128], F32R, tag="o128r")
            nc.vector.tensor_copy(ones128r[:], ones_f32[0:1, :])
            eps_sb = p_const.tile([1, 1], F32, tag="eps")
            nc.vector.memset(eps_sb[:], EPS)

            # v-ones columns in kv_send (written once; col 64/129 of each
            # [128,130] v block stays 1.0 across layers)
            vpart = kv_send[VOFF:NRANK].rearrange(
                "(s sl hp p c) -> s sl hp p c", s=2, sl=2, hp=8, p=128, c=130)
            for s in range(2):
                for sl in range(2):
                    for hp in range(8):
                        nc.sync.dma_start(vpart[s, sl, hp, :, 64:65],
                                          ones_bf[:, 0:1])
                        nc.sync.dma_start(vpart[s, sl, hp, :, 129:130],
                                          ones_bf[:, 1:2])

            # ---- residual stream x^T (f32, persists) ----
            x_tiles = []
            for k in range(KT):
                xt = p_x.tile([128, TOK], F32, tag="x", name=f"x_{k}")
                nc.sync.dma_start(xt[:], x0t[k])
                x_tiles.append(xt)

            def layernorm(wcol, bcol, name):
                """LN over E of x_tiles -> list of bf16 [128,TOK] tiles."""
                ps_s = ps_row.tile([1, TOK], F32, tag="r", name=f"{name}ps_s")
                ps_q = ps_row.tile([1, TOK], F32, tag="r", name=f"{name}ps_q")
                for k in range(KT):
                    xbk = p_ln.tile([128, TOK], BF16, tag="t",
                                    name=f"{name}xb{k}")
                    nc.vector.tensor_copy(xbk[:], x_tiles[k][:])
                    sqk = p_ln.tile([128, TOK], BF16, tag="t",
                                    name=f"{name}sq{k}")
                    nc.scalar.activation(sqk[:], xbk[:], SQUARE)
                    nc.tensor.matmul(ps_s[:], ones_bf[:, 0:1], xbk[:],
                                     start=(k == 0), stop=(k == KT - 1))
                    nc.tensor.matmul(ps_q[:], ones_bf[:, 0:1], sqk[:],
                                     start=(k == 0), stop=(k == KT - 1))
                mean = p_row.tile([1, TOK], F32, tag="row", name=f"{name}mean")
                nc.scalar.mul(mean[:], ps_s[:], 1.0 / E)
                ex2 = p_row.tile([1, TOK], F32, tag="row", name=f"{name}ex2")
                nc.scalar.mul(ex2[:], ps_q[:], 1.0 / E)
                msq = p_row.tile([1, TOK], F32, tag="row", name=f"{name}msq")
                nc.scalar.activation(msq[:], mean[:], SQUARE)
                var = p_row.tile([1, TOK], F32, tag="row", name=f"{name}var")
                nc.vector.tensor_sub(var[:], ex2[:], msq[:])
                std = p_row.tile([1, TOK], F32, tag="row", name=f"{name}std")
                nc.scalar.activation(std[:], var[:], SQRT, bias=eps_sb[:])
                rstd = p_row.tile([1, TOK], F32, tag="row", name=f"{name}rstd")
                nc.vector.reciprocal(rstd[:], std[:])
                # broadcast mean/rstd across partitions via K=1 f32r matmuls
                meanr = p_rowr.tile([1, TOK], F32R, tag="rr", name=f"{name}meanr")
                nc.vector.tensor_copy(meanr[:], mean[:])
                rstdr = p_rowr.tile([1, TOK], F32R, tag="rr", name=f"{name}rstdr")
                nc.vector.tensor_copy(rstdr[:], rstd[:])
                mb = ps_mm.tile([128, TOK], F32, tag="mm", name=f"{name}mb")
                nc.tensor.matmul(mb[:], ones128r[:], meanr[:], start=True,
                                 stop=True)
                rb = ps_mm.tile([128, TOK], F32, tag="mm", name=f"{name}rb")
                nc.tensor.matmul(rb[:], ones128r[:], rstdr[:], start=True,
                                 stop=True)
                out = []
                for k in range(KT):
                    t1 = p_f32.tile([128, TOK], F32, tag="f", name=f"{name}t1_{k}")
                    nc.vector.tensor_sub(t1[:], x_tiles[k][:], mb[:])
                    t2 = p_f32.tile([128, TOK], F32, tag="f", name=f"{name}t2_{k}")
                    nc.vector.tensor_mul(t2[:], t1[:], rb[:])
                    h = p_h.tile([128, TOK], BF16, tag="h", name=f"{name}h{k}")
                    nc.scalar.activation(h[:], t2[:], IDENT,
                                         bias=bcol[k], scale=wcol[k])
                    out.append(h)
                return out

            for layer in range(n_layers):
                lp = p_lnp.tile([128, 80], F32, tag="lnp", name=f"lnp{layer}")
                nc.sync.dma_start(lp[:], lnp[layer])

                h1 = layernorm([lp[:, C_LN1W + k:C_LN1W + k + 1] for k in range(KT)],
                               [lp[:, C_LN1B + k:C_LN1B + k + 1] for k in range(KT)],
                               f"L{layer}ln1")

                # ---- K, V projections first (feed the AllGather early) ----
                wk_sb = []
                for k in range(KT):
                    t = p_w.tile([128, E], BF16, tag="w", name=f"L{layer}wk{k}")
                    nc.sync.dma_start(t[:], wk[layer, k * 128:(k + 1) * 128, :])
                    wk_sb.append(t)
                kT_sb = []
                for hp in range(8):
                    ps = ps_mm.tile([128, TOK], F32, tag="mm",
                                    name=f"L{layer}psk{hp}")
                    for k in range(KT):
                        nc.tensor.matmul(ps[:], wk_sb[k][:, hp * 128:(hp + 1) * 128],
                                         h1[k][:], start=(k == 0),
                                         stop=(k == KT - 1))
                    t = p_qk.tile([128, TOK], BF16, tag="qk",
                                  name=f"L{layer}kT{hp}")
                    nc.vector.tensor_copy(t[:], ps[:])
                    kT_sb.append(t)
                    for s in range(2):
                        nc.sync.dma_start(
                            kv_send[_koff(s, hp):_koff(s, hp) + 32768]
                            .rearrange("(p c) -> p c", p=128),
                            t[:, s * 256:(s + 1) * 256])

                wv_sb = []
                for k in range(KT):
                    t = p_w.tile([128, E], BF16, tag="w", name=f"L{layer}wv{k}")
                    nc.sync.dma_start(t[:], wv[layer, k * 128:(k + 1) * 128, :])
                    wv_sb.append(t)
                vloc = {}
                for s in range(2):
                    for sl in range(2):
                        tc_idx = s * 2 + sl  # token chunk index in TOK cols
                        for g in range(4):  # head groups of 4
                            ps = ps_mm.tile([128, 256], F32, tag="mm",
                                            name=f"L{layer}psv{tc_idx}_{g}")
                            for k in range(KT):
                                nc.tensor.matmul(
                                    ps[:],
                                    h1[k][:, tc_idx * 128:(tc_idx + 1) * 128],
                                    wv_sb[k][:, g * 256:(g + 1) * 256],
                                    start=(k == 0), stop=(k == KT - 1))
                            for hh in range(2):  # head pairs within group
                                hp = g * 2 + hh
                                vv = p_vloc.tile([128, 130], BF16, tag="vl",
                                                 name=f"L{layer}vv{s}{sl}{hp}")
                                nc.vector.tensor_copy(
                                    vv[:].rearrange("p (h c) -> p h c", h=2)
                                    [:, :, 0:64],
                                    ps[:, hh * 128:(hh + 1) * 128]
                                    .rearrange("p (h c) -> p h c", h=2))
                                nc.vector.memset(
                                    vv[:].rearrange("p (h c) -> p h c", h=2)
                                    [:, :, 64:65], 1.0)
                                vloc[(s, sl, hp)] = vv
                                nc.sync.dma_start(
                                    kv_send[_voff(s, sl, hp):
                                            _voff(s, sl, hp) + 16640]
                                    .rearrange("(p c) -> p c", p=128)
                                    [:, 0:130].rearrange("p (h c) -> p h c", h=2)
                                    [:, :, 0:64],
                                    vv[:].rearrange("p (h c) -> p h c", h=2)
                                    [:, :, 0:64])

                nc.gpsimd.collective_compute(
                    "AllGather", mybir.AluOpType.bypass,
                    replica_groups=[list(range(NCORE))],
                    ins=[kv_send[:].opt()], outs=[kv_gath[:].opt()])

                # ---- Q projection (overlaps the AllGather) ----
                wq_sb = []
                for k in range(KT):
                    t = p_w.tile([128, E], BF16, tag="w", name=f"L{layer}wq{k}")
                    nc.sync.dma_start(t[:], wq[layer, k * 128:(k + 1) * 128, :])
                    wq_sb.append(t)
                qT_sb = []
                for hp in range(8):
                    ps = ps_mm.tile([128, TOK], F32, tag="mm",
                                    name=f"L{layer}psq{hp}")
                    for k in range(KT):
                        nc.tensor.matmul(ps[:], wq_sb[k][:, hp * 128:(hp + 1) * 128],
                                         h1[k][:], start=(k == 0),
                                         stop=(k == KT - 1))
                    t = p_qk.tile([128, TOK], BF16, tag="qk",
                                  name=f"L{layer}qT{hp}")
                    nc.vector.tensor_copy(t[:], ps[:])
                    qT_sb.append(t)

                # ---- attention ----
                oT_sb = []
                for k in range(KT):
                    t = p_oT.tile([128, TOK], BF16, tag="o", name=f"L{layer}oT{k}")
                    oT_sb.append(t)

                kv_rk = kv_gath[:].rearrange("(r e) -> r e", r=NCORE)
                for s in range(2):
                    for hp in range(8):
                        # batched loads: one DMA for all 8 ranks' k, one per
                        # v slot (8 chunks each)
                        ko = _koff(s, hp)
                        kt_all = p_kt.tile([128, 8 * 256], BF16, tag="kt",
                                           name=f"L{layer}kt{s}{hp}")
                        nc.sync.dma_start(
                            kt_all[:].rearrange("p (r c) -> p r c", r=8),
                            kv_rk[:, ko:ko + 32768]
                            .rearrange("r (p c) -> p r c", p=128))
                        vo0 = _voff(s, 0, hp)
                        vt0 = p_vt.tile([128, 8 * 130], BF16, tag="vt",
                                        name=f"L{layer}vt0_{s}{hp}")
                        nc.sync.dma_start(
                            vt0[:].rearrange("p (r c) -> p r c", r=8),
                            kv_rk[:, vo0:vo0 + 16640]
                            .rearrange("r (p c) -> p r c", p=128))
                        vo1 = _voff(s, 1, hp)
                        vt1 = p_vt.tile([128, 8 * 130], BF16, tag="vt",
                                        name=f"L{layer}vt1_{s}{hp}")
                        nc.sync.dma_start(
                            vt1[:].rearrange("p (r c) -> p r c", r=8),
                            kv_rk[:, vo1:vo1 + 16640]
                            .rearrange("r (p c) -> p r c", p=128))

                        def kslc(j, ro):
                            # k^T [64,128] slice for global kv chunk j
                            if j < 8:
                                return kt_all[ro:ro + 64,
                                              j * 256:j * 256 + 128]
                            r = 15 - j
                            return kt_all[ro:ro + 64,
                                          r * 256 + 128:r * 256 + 256]

                        def vslc(j, hh):
                            # v_ext [128,65] slice for global kv chunk j
                            if j < 8:
                                return vt0[:, j * 130 + hh * 65:
                                           j * 130 + hh * 65 + 65]
                            r = 15 - j
                            return vt1[:, r * 130 + hh * 65:
                                       r * 130 + hh * 65 + 65]

                        for hh in range(2):
                            ro = hh * 64
                            qab = qT_sb[hp][ro:ro + 64, s * 256:s * 256 + 256]
                            qa = qT_sb[hp][ro:ro + 64, s * 256:s * 256 + 128]
                            qb = qT_sb[hp][ro:ro + 64,
                                           s * 256 + 128:s * 256 + 256]
                            tp = (ro, 0)
                            nm = f"L{layer}a{s}{hp}{hh}"

                            # AB groups: kv j=0..7, both chunks (N=256);
                            # chunk-a cols bias-masked via maskab
                            e_ab = []
                            for g in range(4):
                                ps = ps_mm.tile([128, 512], F32, tag="mm",
                                                name=f"{nm}psAB{g}")
                                for i in range(2):
                                    nc.tensor.matmul(
                                        ps[:, i * 256:(i + 1) * 256],
                                        kslc(2 * g + i, ro), qab,
                                        start=True, stop=True,
                                        tile_position=tp)
                                tmp = p_f32.tile([128, 512], F32, tag="f",
                                                 name=f"{nm}tmAB{g}")
                                nc.vector.tensor_tensor(
                                    tmp[:], ps[:],
                                    maskab_sb[:, g * 512:(g + 1) * 512],
                                    op=ADD)
                                et = p_E.tile([128, 512], BF16, tag="E",
                                              name=f"{nm}eAB{g}")
                                nc.scalar.activation(et[:], tmp[:], EXPF,
                                                     scale=0.125)
                                e_ab.append(et)
                            # B tail: kv j=8..15, chunk b only (N=128)
                            e_bt = []
                            for g in range(2):
                                ps = ps_mm.tile([128, 512], F32, tag="mm",
                                                name=f"{nm}psBt{g}")
                                for i in range(4):
                                    j = 8 + g * 4 + i
                                    nc.tensor.matmul(
                                        ps[:, i * 128:(i + 1) * 128],
                                        kslc(j, ro), qb,
                                        start=True, stop=True,
                                        tile_position=tp)
                                tmp = p_f32.tile([128, 512], F32, tag="f",
                                                 name=f"{nm}tmBt{g}")
                                nc.vector.tensor_tensor(
                                    tmp[:], ps[:],
                                    maskbt_sb[:, g * 512:(g + 1) * 512],
                                    op=ADD)
                                et = p_E.tile([128, 512], BF16, tag="E",
                                              name=f"{nm}eBt{g}")
                                nc.scalar.activation(et[:], tmp[:], EXPF,
                                                     scale=0.125)
                                e_bt.append(et)
                            # diagonal blocks from LOCAL k/q (chunk a & b)
                            psd = ps_mm.tile([128, 256], F32, tag="mm",
                                             name=f"{nm}psD")
                            nc.tensor.matmul(
                                psd[:, 0:128],
                                kT_sb[hp][ro:ro + 64, s * 256:s * 256 + 128],
                                qa, start=True, stop=True, tile_position=tp)
                            nc.tensor.matmul(
                                psd[:, 128:256],
                                kT_sb[hp][ro:ro + 64,
                                          s * 256 + 128:s * 256 + 256],
                                qb, start=True, stop=True, tile_position=tp)
                            tmpd = p_f32.tile([128, 256], F32, tag="f",
                                              name=f"{nm}tmD")
                            nc.vector.tensor_tensor(tmpd[:], psd[:], tri_sb[:],
                                                    op=ADD)
                            etd = p_E.tile([128, 256], BF16, tag="E",
                                           name=f"{nm}eD")
                            nc.scalar.activation(etd[:], tmpd[:], EXPF,
                                                 scale=0.125)

                            # o accumulation [65, 256]: cols 0:128 chunk a,
                            # 128:256 chunk b
                            po = ps_acc.tile([65, 256], F32, tag="acc",
                                             name=f"{nm}po")
                            for j in range(8):   # both chunks at once, N=256
                                nc.tensor.matmul(
                                    po[:], vslc(j, hh),
                                    e_ab[j // 2][:, (j % 2) * 256:
                                                 (j % 2) * 256 + 256],
                                    start=(j == 0), stop=False)
                            for j in range(8, 16):   # chunk b only, N=128
                                nc.tensor.matmul(
                                    po[:, 128:256], vslc(j, hh),
                                    e_bt[(j - 8) // 4][:, ((j - 8) % 4) * 128:
                                                       ((j - 8) % 4) * 128 + 128],
                                    start=False, stop=False)
                            nc.tensor.matmul(
                                po[:, 0:128],
                                vloc[(s, 0, hp)][:, hh * 65:hh * 65 + 65],
                                etd[:, 0:128], start=False, stop=True)
                            nc.tensor.matmul(
                                po[:, 128:256],
                                vloc[(s, 1, hp)][:, hh * 65:hh * 65 + 65],
                                etd[:, 128:256], start=False, stop=True)

                            # normalize: o / sumexp (row 64)
                            rrow = p_row.tile([1, 256], F32, tag="row",
                                              name=f"{nm}rrow")
                            nc.vector.reciprocal(rrow[:], po[64:65, :])
                            rrr = p_rowr.tile([1, 256], F32R, tag="rr",
                                              name=f"{nm}rrr")
                            nc.vector.tensor_copy(rrr[:], rrow[:])
                            prb = ps_row.tile([64, 256], F32, tag="r",
                                              name=f"{nm}prb")
                            nc.tensor.matmul(prb[:], ones64r[:], rrr[:],
                                             start=True, stop=True)
                            rbs = p_f32.tile([64, 256], F32, tag="f",
                                             name=f"{nm}rbs")
                            nc.scalar.copy(rbs[:], prb[:])
                            h = hp * 2 + hh
                            dst = oT_sb[h // 2][(h % 2) * 64:(h % 2) * 64 + 64,
                                                s * 256:s * 256 + 256]
                            nc.vector.tensor_mul(dst, po[0:64, :], rbs[:])

                # ---- output projection + residual ----
                wo_sb = []
                for k in range(KT):
                    t = p_w.tile([128, E], BF16, tag="w", name=f"L{layer}wo{k}")
                    nc.sync.dma_start(t[:], wo[layer, k * 128:(k + 1) * 128, :])
                    wo_sb.append(t)
                for eo in range(KT):
                    ps = ps_mm.tile([128, TOK], F32, tag="mm",
                                    name=f"L{layer}psd{eo}")
                    for k in range(KT):
                        nc.tensor.matmul(ps[:],
                                         wo_sb[k][:, eo * 128:(eo + 1) * 128],
                                         oT_sb[k][:], start=(k == 0),
                                         stop=(k == KT - 1))
                    ds = p_f32.tile([128, TOK], F32, tag="f",
                                    name=f"L{layer}ds{eo}")
                    nc.scalar.activation(ds[:], ps[:], IDENT,
                                         bias=lp[:, C_BO + eo:C_BO + eo + 1])
                    nc.vector.tensor_add(x_tiles[eo][:], x_tiles[eo][:], ds[:])

                # ---- FFN ----
                h2 = layernorm([lp[:, C_LN2W + k:C_LN2W + k + 1] for k in range(KT)],
                               [lp[:, C_LN2B + k:C_LN2B + k + 1] for k in range(KT)],
                               f"L{layer}ln2")
                m_sb = []
                for q in range(4):
                    w1_sb = []
                    for k in range(KT):
                        t = p_w.tile([128, E], BF16, tag="w",
                                     name=f"L{layer}w1_{q}_{k}")
                        nc.sync.dma_start(
                            t[:], w1[layer, k * 128:(k + 1) * 128,
                                     q * 1024:(q + 1) * 1024])
                        w1_sb.append(t)
                    for mo in range(8):
                        mi = q * 8 + mo
                        ps = ps_mm.tile([128, TOK], F32, tag="mm",
                                        name=f"L{layer}psm{mi}")
                        for k in range(KT):
                            nc.tensor.matmul(
                                ps[:], w1_sb[k][:, mo * 128:(mo + 1) * 128],
                                h2[k][:], start=(k == 0), stop=(k == KT - 1))
                        mt = p_mT.tile([128, TOK], BF16, tag="m",
                                       name=f"L{layer}m{mi}")
                        nc.scalar.activation(
                            mt[:], ps[:], RELU,
                            bias=lp[:, C_B1 + mi:C_B1 + mi + 1])
                        m_sb.append(mt)
                for eop in range(4):
                    pse = [ps_mm.tile([128, TOK], F32, tag="mm",
                                      name=f"L{layer}pse{eop}_{u}")
                           for u in range(2)]
                    for kg in range(4):   # batches of 8 k-tiles per DMA
                        t = p_w2.tile([128, 8 * 256], BF16, tag="w2",
                                      name=f"L{layer}w2_{eop}_{kg}")
                        nc.sync.dma_start(
                            t[:].rearrange("p (kk c) -> p kk c", kk=8),
                            w2[layer, kg * 1024:(kg + 1) * 1024,
                               eop * 256:(eop + 1) * 256]
                            .rearrange("(kk p) c -> p kk c", p=128))
                        for kk in range(8):
                            k = kg * 8 + kk
                            for u in range(2):
                                nc.tensor.matmul(
                                    pse[u][:],
                                    t[:, kk * 256 + u * 128:
                                      kk * 256 + (u + 1) * 128],
                                    m_sb[k][:], start=(k == 0),
                                    stop=(k == 31))
                    for u in range(2):
                        eo = eop * 2 + u
                        ds = p_f32.tile([128, TOK], F32, tag="f",
                                        name=f"L{layer}ds2{eo}")
                        nc.scalar.activation(
                            ds[:], pse[u][:], IDENT,
                            bias=lp[:, C_B2 + eo:C_B2 + eo + 1])
                        nc.vector.tensor_add(x_tiles[eo][:], x_tiles[eo][:],
                                             ds[:])

            if debug_xt:
                for k in range(KT):
                    nc.sync.dma_start(dbg_o[k], x_tiles[k][:])

            # ---- final LN + AllGather of xf^T ----
            xf = layernorm([lnfp_sb[:, k:k + 1] for k in range(KT)],
                           [lnfp_sb[:, 8 + k:8 + k + 1] for k in range(KT)],
                           "lnf")
            for k in range(KT):
                nc.sync.dma_start(
                    xf_send[k * 128 * TOK:(k + 1) * 128 * TOK]
                    .rearrange("(p c) -> p c", p=128), xf[k][:])
            nc.gpsimd.collective_compute(
                "AllGather", mybir.AluOpType.bypass,
                replica_groups=[list(range(NCORE))],
                ins=[xf_send[:].opt()], outs=[xf_gath[:].opt()])

            # ---- lm_b broadcast tiles ----
            lmb_bc = []
            for n in range(8):
                row = p_row.tile([1, 500], F32, tag="row", name=f"lmbrow{n}")
                nc.sync.dma_start(row[:], lmb[n:n + 1, :])
                rowr = p_rowr.tile([1, 500], F32R, tag="rr", name=f"lmbr{n}")
                nc.vector.tensor_copy(rowr[:], row[:])
                pb = ps_mm.tile([128, 500], F32, tag="mm", name=f"lmbps{n}")
                nc.tensor.matmul(pb[:], ones128r[:], rowr[:], start=True,
                                 stop=True)
                t = p_lmbbc.tile([128, 500], BF16, tag="bb", name=f"lmbbc{n}")
                nc.vector.tensor_copy(t[:], pb[:])
                lmb_bc.append(t)

            # ---- LM head: logits[tok, vocab-shard] ----
            sacc = [p_sacc.tile([128, 8], F32, tag="sa", name=f"sacc{m}")
                    for m in range(32)]
            for n in range(8):
                lw_sb = []
                for k in range(KT):
                    t = p_w.tile([128, 500], BF16, tag="w", name=f"lw{n}_{k}")
                    nc.sync.dma_start(
                        t[:], lmw[k * 128:(k + 1) * 128,
                                  n * 500:(n + 1) * 500])
                    lw_sb.append(t)
                for r in range(NCORE):
                    xfa = p_xfa.tile([128, KT * TOK], BF16, tag="xfa",
                                     name=f"xfg{n}_{r}")
                    off = r * KT * 128 * TOK
                    nc.sync.dma_start(
                        xfa[:].rearrange("p (k c) -> p k c", k=KT),
                        xf_gath[off:off + KT * 128 * TOK]
                        .rearrange("(k p c) -> p k c", k=KT, p=128))
                    for ml in range(4):
                        m = r * 4 + ml
                        ps = ps_mm.tile([128, 500], F32, tag="mm",
                                        name=f"lg{n}_{m}")
                        for k in range(KT):
                            nc.tensor.matmul(
                                ps[:],
                                xfa[:, k * TOK + ml * 128:
                                    k * TOK + (ml + 1) * 128],
                                lw_sb[k][:], start=(k == 0),
                                stop=(k == KT - 1))
                        lg = p_f32.tile([128, 500], F32, tag="f",
                                        name=f"lgs{n}_{m}")
                        nc.vector.tensor_tensor(lg[:], ps[:], lmb_bc[n][:],
                                                op=ADD)
                        nc.sync.dma_start(
                            logits_o[m * 128:(m + 1) * 128,
                                     n * 500:(n + 1) * 500], lg[:])
                        # exp row-sums for the loss
                        junk = p_f32.tile([128, 500], F32, tag="f",
                                          name=f"ej{n}_{m}")
                        nc.scalar.activation(junk[:], lg[:], EXPF,
                                             accum_out=sacc[m][:, n:n + 1])
            # assemble sexp: for each m, sum the 8 per-shard-chunk sums
            for m in range(32):
                srow = p_row.tile([128, 1], F32, tag="row", name=f"srow{m}")
                nc.vector.reduce_sum(srow[:], sacc[m][:],
                                     axis=mybir.AxisListType.X)
                nc.sync.dma_start(sexp_o[m:m + 1, :].rearrange("a b -> (a b)"),
                                  srow[:])
    nc.compile()
    return nc


_NC_CACHE = {}


def _get_nc(n_layers=L, debug_xt=False):
    key = (n_layers, debug_xt)
    if key not in _NC_CACHE:
        _NC_CACHE[key] = build(n_layers, debug_xt)
    return _NC_CACHE[key]


def _pos_encoding(t, e):
    pos = np.arange(t, dtype=np.float32)[:, None]
    div = np.exp(np.arange(0, e, 2, dtype=np.float32)
                 * np.float32(-math.log(10000.0) / e))
    ang = pos * div
    return np.stack([np.sin(ang), np.cos(ang)], axis=-1).reshape(t, e) \
        .astype(np.float32)


def _chunks_of(r):
    return (r, NCH - 1 - r)


def _token_perm():
    """gathered row g -> flat (b*T + t) index."""
    perm = np.empty(NCORE * TOK, dtype=np.int64)
    g = 0
    for r in range(NCORE):
        ca, cb = _chunks_of(r)
        for s in range(B):
            for c in (ca, cb):
                for p in range(128):
                    perm[g] = s * T + c * 128 + p
                    g += 1
    return perm


def _host_inputs(inputs, n_layers=L):
    bf = ml_dtypes.bfloat16
    tok_emb = np.asarray(inputs["tok_emb"], np.float32)
    idx = np.asarray(inputs["idx"])
    x0 = tok_emb[idx] + _pos_encoding(T, E)[None, :, :]   # [B, T, E] f32

    shared = {
        "wq": np.ascontiguousarray(np.asarray(inputs["wq"], np.float32)).astype(bf),
        "wk": np.ascontiguousarray(np.asarray(inputs["wk"], np.float32)).astype(bf),
        "wv": np.ascontiguousarray(np.asarray(inputs["wv"], np.float32)).astype(bf),
        "wo": np.ascontiguousarray(np.asarray(inputs["wo"], np.float32)).astype(bf),
        "w1": np.ascontiguousarray(np.asarray(inputs["w1"], np.float32)).astype(bf),
        "w2": np.ascontiguousarray(np.asarray(inputs["w2"], np.float32)).astype(bf),
    }
    lnp = np.zeros((L, 128, 80), np.float32)
    for l in range(L):
        for base, nm in ((C_LN1W, "ln1_w"), (C_LN1B, "ln1_b"),
                         (C_LN2W, "ln2_w"), (C_LN2B, "ln2_b"),
                         (C_BO, "bo"), (C_B2, "b2")):
            v = np.asarray(inputs[nm], np.float32)[l]          # [1024]
            lnp[l, :, base:base + 8] = v.reshape(8, 128).T
        v = np.asarray(inputs["b1"], np.float32)[l]            # [4096]
        lnp[l, :, C_B1:C_B1 + 32] = v.reshape(32, 128).T
    shared["lnp"] = lnp
    lnfp = np.zeros((128, 16), np.float32)
    lnfp[:, 0:8] = np.asarray(inputs["lnf_w"], np.float32).reshape(8, 128).T
    lnfp[:, 8:16] = np.asarray(inputs["lnf_b"], np.float32).reshape(8, 128).T
    shared["lnfp"] = lnfp
    tri = np.where(np.arange(128)[:, None] <= np.arange(128)[None, :],
                   np.float32(0.0), np.float32(NEG))
    shared["tri2"] = np.concatenate([tri, tri], axis=1).astype(bf)

    lm_w = np.ascontiguousarray(np.asarray(inputs["lm_w"], np.float32)).astype(bf)
    lm_b = np.asarray(inputs["lm_b"], np.float32)

    in_maps = []
    for r in range(NCORE):
        ca, cb = _chunks_of(r)
        cols = []
        for s in range(B):
            for c in (ca, cb):
                cols.append(x0[s, c * 128:(c + 1) * 128, :])   # [128, E]
        xr = np.concatenate(cols, axis=0).T                    # [E, TOK]
        x0t = np.ascontiguousarray(xr.reshape(KT, 128, TOK))

        # maskab: group g covers kv j=2g,2g+1 as [j a|j b|j' a|j' b]
        maskab = np.zeros((128, 2048), np.float32)
        for j in range(8):
            if j >= ca:                       # chunk a diag/future -> masked
                maskab[:, j * 256:j * 256 + 128] = NEG
        maskbt = np.zeros((128, 1024), np.float32)
        for j in range(8, 16):
            if j >= cb:                       # chunk b diag/future -> masked
                maskbt[:, (j - 8) * 128:(j - 7) * 128] = NEG

        m = dict(shared)
        m["x0t"] = x0t
        m["maskab"] = maskab.astype(bf)
        m["maskbt"] = maskbt.astype(bf)
        m["lmw"] = np.ascontiguousarray(lm_w[:, r * VS:(r + 1) * VS])
        m["lmb"] = np.ascontiguousarray(
            lm_b[r * VS:(r + 1) * VS].reshape(8, 500))
        in_maps.append(m)
    return in_maps


def _assemble(results, inputs):
    perm = _token_perm()
    logits = np.empty((B * T, V), np.float32)
    sexp_g = np.zeros(NCORE * TOK, np.float64)
    for r in range(NCORE):
        logits[perm, r * VS:(r + 1) * VS] = results[r]["logits_s"]
        sexp_g += results[r]["sexp"].reshape(-1).astype(np.float64)
    lse_g = np.log(sexp_g)                                  # gathered order
    lse = np.empty(B * T, np.float64)
    lse[perm] = lse_g
    targets = np.asarray(inputs["targets"]).reshape(-1)
    tgt_logit = logits[np.arange(B * T), targets].astype(np.float64)
    loss = np.float32(np.mean(lse - tgt_logit))
    return logits, loss


def kernel(**inputs):
    nc = _get_nc()
    in_maps = _host_inputs(inputs)
    res = run_bass_kernel_spmd(nc, in_maps, list(range(NCORE)))
    return _assemble(res.results, inputs)
